# revision 9
# baseline (speedup 1.0000x reference)
"""AttentionBlock (GroupNorm + single-head self-attention + residual) on Trainium2.

Reference computation (per sample, C=256 channels, N=H*W=1024 positions):
    h   = GroupNorm32(x) * gn_w + gn_b
    q   = wq @ h + bq;  k = wk @ h + bk;  v = wv @ h + bv      (1x1 convs)
    att = softmax((q^T k) * C^-0.5)                            [N, N]
    out = x + wo @ (att-weighted v) + bo

Sharding: data-parallel over batch B=32 across 8 NeuronCores (4 samples each).
Weights are replicated; everything per-sample stays on-chip.

Key algebraic rearrangements (exact, up to fp reassociation):
  * wo is folded into the v projection: vo = (wo@wv) @ h + (wo@bv + bo).
    Since softmax rows sum to 1, the constant (wo@bv + bo) passes through
    attention unchanged, so the final 1x1 conv disappears.
  * softmax is computed without max-subtraction (att*scale has |x| < ~1 here),
    and the division by the row sum is applied after the att @ vo matmul.
  * attention is computed transposed (attT[m, n]) so both bmms contract over
    the partition dimension; row sums are accumulated tile-wise on VectorE,
    then collapsed across partitions with a single M=1 ones-matmul per chunk.

Pipeline: all 4 samples' GroupNorm statistics are computed in a prologue so
the PE stream (projections -> attT -> att@vo) never waits on the
DVE->PE->ACT stats chain at sample boundaries (keeps the PE HAM-warm and
avoids per-sample Sqrt/Exp activation-table thrash).
"""

import sys

import ml_dtypes
import numpy as np

for _p in ("/opt/trn_rl_repo",):
    if _p not in sys.path:
        sys.path.insert(0, _p)

import concourse.bacc as bacc
import concourse.bass as bass
import concourse.tile as tile
from concourse import mybir
from concourse.bass_utils import run_bass_kernel_spmd

P = 128
B = 32
B_LOC = 4           # samples per core
C = 256
N = 1024            # H*W
CI = C // P         # 2 channel chunks (contraction side)
NT = N // P         # 8 spatial 128-tiles
FD = 512            # matmul free-dim chunk (one PSUM bank of fp32)
NF = N // FD        # 2 free chunks
G = 32              # groups
EPS = 1e-5
SCALE = float(C) ** -0.5
F32 = mybir.dt.float32
F32R = mybir.dt.float32r
BF16 = mybir.dt.bfloat16
AF = mybir.ActivationFunctionType
OP = mybir.AluOpType


def _r(ap):
    """View an fp32 AP as float32r for full-rate PE matmuls."""
    return ap.bitcast(F32R)


def build_nc():
    nc = bacc.Bacc("TRN2", debug=False, num_devices=8, enable_asserts=False)

    x_d = nc.dram_tensor("x", [B_LOC, C, N], F32, kind="ExternalInput").ap()
    wq_d = nc.dram_tensor("wqT", [C, C], BF16, kind="ExternalInput").ap()
    wk_d = nc.dram_tensor("wkT", [C, C], BF16, kind="ExternalInput").ap()
    wvo_d = nc.dram_tensor("wvoT", [C, C], BF16, kind="ExternalInput").ap()
    bq_d = nc.dram_tensor("bq", [C], F32, kind="ExternalInput").ap()
    bk_d = nc.dram_tensor("bk", [C], F32, kind="ExternalInput").ap()
    bvo_d = nc.dram_tensor("bvo", [C], F32, kind="ExternalInput").ap()
    gnw_d = nc.dram_tensor("gnw", [C], F32, kind="ExternalInput").ap()
    gnb_d = nc.dram_tensor("gnb", [C], F32, kind="ExternalInput").ap()
    gsel_d = nc.dram_tensor("gsel", [CI, P, G], F32, kind="ExternalInput").ap()
    ones_d = nc.dram_tensor("ones", [P, 1], F32R, kind="ExternalInput").ap()
    bsel_d = nc.dram_tensor("bsel", [CI, G, P], F32, kind="ExternalInput").ap()
    out_d = nc.dram_tensor("out", [B_LOC, C, N], F32, kind="ExternalOutput").ap()

    x_r = x_d.rearrange("b (ci p) n -> b p ci n", p=P)
    out_r = out_d.rearrange("b (co p) n -> b p co n", p=P)

    with tile.TileContext(nc) as tc:
        with (
            tc.tile_pool(name="const", bufs=1) as const,
            tc.tile_pool(name="xp", bufs=B_LOC) as xp,
            tc.tile_pool(name="hp", bufs=2) as hp,
            tc.tile_pool(name="qkp", bufs=2) as qkp,
            tc.tile_pool(name="vop", bufs=2) as vop,
            tc.tile_pool(name="attp", bufs=2) as attp,
            tc.tile_pool(name="outp", bufs=1) as outp,
            tc.tile_pool(name="accp", bufs=1) as accp,
            tc.tile_pool(name="smallp", bufs=2) as smallp,
            tc.tile_pool(name="stp", bufs=B_LOC) as stp,
            tc.tile_pool(name="rp", bufs=1) as rp,
            tc.tile_pool(name="psA", bufs=2, space="PSUM") as psA,  # proj/vo
            tc.tile_pool(name="psB", bufs=2, space="PSUM") as psB,  # att
            tc.tile_pool(name="psC", bufs=2, space="PSUM") as psC,  # out
        ):
            # ---------------- constants (loaded once) ----------------
            wq_sb = const.tile([P, CI, C], BF16, tag="wq")
            nc.scalar.dma_start(wq_sb, wq_d.rearrange("(ci p) o -> p ci o", p=P))
            wk_sb = const.tile([P, CI, C], BF16, tag="wk")
            nc.scalar.dma_start(wk_sb, wk_d.rearrange("(ci p) o -> p ci o", p=P))
            wvo_sb = const.tile([P, CI, C], BF16, tag="wvo")
            nc.scalar.dma_start(wvo_sb, wvo_d.rearrange("(ci p) o -> p ci o", p=P))

            bq_sb = const.tile([P, CI], F32, tag="bq")
            nc.scalar.dma_start(bq_sb, bq_d.rearrange("(co p) -> p co", p=P))
            bk_sb = const.tile([P, CI], F32, tag="bk")
            nc.scalar.dma_start(bk_sb, bk_d.rearrange("(co p) -> p co", p=P))
            gnw_sb = const.tile([P, CI], F32, tag="gnw")
            nc.scalar.dma_start(gnw_sb, gnw_d.rearrange("(ci p) -> p ci", p=P))
            gnb_sb = const.tile([P, CI], F32, tag="gnb")
            nc.scalar.dma_start(gnb_sb, gnb_d.rearrange("(ci p) -> p ci", p=P))

            bvo_sb = const.tile([P, CI], F32, tag="bvo")
            nc.scalar.dma_start(bvo_sb, bvo_d.rearrange("(co p) -> p co", p=P))

            gsel_sb = const.tile([P, CI, G], F32, tag="gsel")
            nc.scalar.dma_start(gsel_sb, gsel_d.rearrange("ci p g -> p ci g"))
            bsel_sb = const.tile([G, CI, P], F32, tag="bsel")
            nc.scalar.dma_start(bsel_sb, bsel_d.rearrange("ci g c -> g ci c"))

            ones_sb = const.tile([P, 1], F32R, tag="ones")
            nc.scalar.dma_start(ones_sb, ones_d)
            eps_sb = const.tile([P, 1], F32, tag="eps")
            nc.vector.memset(eps_sb, EPS)

            # -------- prologue: load x + GroupNorm stats for ALL samples --------
            x_sbs = []
            st_sbs = []
            for s in range(B_LOC):
                x_sb = xp.tile([P, CI, N], F32, tag="x")
                nc.sync.dma_start(x_sb[:, 0, :], x_r[s][:, 0, :])
                nc.gpsimd.dma_start(x_sb[:, 1, :], x_r[s][:, 1, :])
                x_sbs.append(x_sb)

            for s in range(B_LOC):
                x_sb = x_sbs[s]
                # per-channel (mean, var, mean^2) -> st3 [P, CI, 3]
                st3 = smallp.tile([P, CI, 3], F32, tag="st3")
                for ci in range(CI):
                    bnst = smallp.tile([P, 2, 6], F32, tag="bnst")
                    for sub in range(2):
                        nc.vector.bn_stats(
                            out=bnst[:, sub, :],
                            in_=x_sb[:, ci, sub * 512:(sub + 1) * 512],
                        )
                    nc.vector.bn_aggr(out=st3[:, ci, 0:2], in_=bnst)
                    nc.vector.tensor_mul(st3[:, ci, 2:3], st3[:, ci, 0:1],
                                         st3[:, ci, 0:1])
                # group-pooled: [G, 3] = (mean_g, E[var_c], E[m_c^2]) per group
                gstat_ps = psA.tile([G, 3], F32, tag="mm")
                for ci in range(CI):
                    nc.tensor.matmul(gstat_ps, lhsT=gsel_sb[:, ci, :],
                                     rhs=st3[:, ci, :],
                                     start=(ci == 0), stop=(ci == CI - 1))
                grp = smallp.tile([G, 2], F32, tag="grp")     # (mean_g, rstd_g)
                gtmp = smallp.tile([G, 2], F32, tag="gtmp")
                gst = smallp.tile([G, 3], F32, tag="gst")
                nc.vector.tensor_copy(gst, gstat_ps)
                nc.vector.tensor_add(gtmp[:, 0:1], gst[:, 1:2], gst[:, 2:3])
                nc.vector.tensor_mul(gtmp[:, 1:2], gst[:, 0:1], gst[:, 0:1])
                nc.vector.tensor_sub(gtmp[:, 0:1], gtmp[:, 0:1], gtmp[:, 1:2])
                nc.vector.tensor_copy(grp[:, 0:1], gst[:, 0:1])
                nc.scalar.activation(out=gtmp[:, 1:2], in_=gtmp[:, 0:1],
                                     func=AF.Sqrt, bias=eps_sb[:G], scale=1.0)
                nc.vector.reciprocal(grp[:, 1:2], gtmp[:, 1:2])

                # broadcast group -> channels; per-channel scale/shift (s_c, t_c)
                st = stp.tile([P, CI, 2], F32, tag="st")
                for ci in range(CI):
                    chan_ps = psA.tile([P, 2], F32, tag="mm")
                    nc.tensor.matmul(chan_ps, lhsT=bsel_sb[:, ci, :], rhs=grp,
                                     start=True, stop=True)
                    nc.vector.tensor_mul(st[:, ci, 0:1], chan_ps[:, 1:2],
                                         gnw_sb[:, ci:ci + 1])
                    nc.vector.tensor_mul(st[:, ci, 1:2], chan_ps[:, 0:1],
                                         st[:, ci, 0:1])
                    nc.vector.tensor_sub(st[:, ci, 1:2], gnb_sb[:, ci:ci + 1],
                                         st[:, ci, 1:2])
                st_sbs.append(st)

            def compute_h(s):
                """h = x * s_c + t_c on ScalarE, written as f32r."""
                h_sb = hp.tile([P, CI, N], BF16, tag="h")
                for ci in range(CI):
                    nc.scalar.activation(out=h_sb[:, ci, :], in_=x_sbs[s][:, ci, :],
                                         func=AF.Identity,
                                         bias=st_sbs[s][:, ci, 1:2],
                                         scale=st_sbs[s][:, ci, 0:1])
                return h_sb

            h_next = compute_h(0)

            # ---------------- per-sample main pipeline ----------------
            for s in range(B_LOC):
                x_sb = x_sbs[s]
                h_sb = h_next

                # -- projections: qT/k in [C, N] layout, vo in [N, C] layout --
                qT_sb = qkp.tile([P, CI, N], BF16, tag="qT")
                k_sb = qkp.tile([P, CI, N], BF16, tag="k")
                for (w_sb, b_sb, dst) in ((wq_sb, bq_sb, qT_sb),
                                          (wk_sb, bk_sb, k_sb)):
                    for co in range(CI):
                        for nf in range(NF):
                            ps = psA.tile([P, FD], F32, tag="mm")
                            for ci in range(CI):
                                nc.tensor.matmul(
                                    ps,
                                    lhsT=w_sb[:, ci, co * P:(co + 1) * P],
                                    rhs=h_sb[:, ci, nf * FD:(nf + 1) * FD],
                                    start=(ci == 0), stop=(ci == CI - 1))
                            nc.scalar.activation(
                                out=dst[:, co, nf * FD:(nf + 1) * FD], in_=ps,
                                func=AF.Identity, bias=b_sb[:, co:co + 1],
                                scale=1.0)

                vo_sb = vop.tile([P, NT, C], BF16, tag="vo")
                for nt in range(NT):
                    ps = psA.tile([P, FD], F32, tag="mm")
                    for ci in range(CI):
                        nc.tensor.matmul(
                            ps[:, 0:C],
                            lhsT=h_sb[:, ci, nt * P:(nt + 1) * P],
                            rhs=wvo_sb[:, ci, :],
                            start=(ci == 0), stop=(ci == CI - 1))
                    nc.vector.tensor_copy(vo_sb[:, nt, :], ps[:, 0:C])

                # -- attT = k^T q, exp on the fly (ScalarE) --
                ax_sb = attp.tile([P, NT, N], BF16, tag="ax")
                for mt in range(NT):
                    ps = psB.tile([P, N], F32, tag="att")
                    for nf in range(NF):
                        for ci in range(CI):
                            nc.tensor.matmul(
                                ps[:, nf * FD:(nf + 1) * FD],
                                lhsT=k_sb[:, ci, mt * P:(mt + 1) * P],
                                rhs=qT_sb[:, ci, nf * FD:(nf + 1) * FD],
                                start=(ci == 0), stop=(ci == CI - 1))
                    nc.scalar.activation(
                        out=ax_sb[:, mt, :], in_=ps,
                        func=AF.Exp, bias=0.0, scale=SCALE)

                # h for the NEXT sample: emitted here so ScalarE computes it
                # while the PE streams this sample's att/out matmuls.
                if s + 1 < B_LOC:
                    h_next = compute_h(s + 1)

                # -- partial softmax row sums on VectorE (tile-wise adds) --
                acc = accp.tile([P, N], F32R, tag="acc")
                nc.vector.tensor_add(acc, ax_sb[:, 0, :], ax_sb[:, 1, :])
                for mt in range(2, NT):
                    nc.vector.tensor_add(acc, acc, ax_sb[:, mt, :])

                # -- out_pre[c, n] = sum_m vo[m, c] * ax[m, n] --
                # First out-matmul group streams while the acc adds finish;
                # the sums collapse + reciprocal + broadcast are emitted right
                # after it so the merge inputs are ready before the remaining
                # groups retire.
                out_sb = outp.tile([P, CI, N], F32, tag="out")
                po_tiles = []

                def out_group(co, nf):
                    po = psC.tile([P, FD], F32, tag="o")
                    for mt in range(NT):
                        nc.tensor.matmul(
                            po,
                            lhsT=vo_sb[:, mt, co * P:(co + 1) * P],
                            rhs=ax_sb[:, mt, nf * FD:(nf + 1) * FD],
                            start=(mt == 0), stop=(mt == NT - 1))
                    po_tiles.append((co, nf, po))

                out_group(0, 0)

                # -- collapse row sums across partitions; reciprocal; broadcast --
                r_row = rp.tile([1, N], F32, tag="rrow")
                for nf in range(NF):
                    sp = psA.tile([P, FD], F32, tag="mm")
                    nc.tensor.matmul(sp[0:1, :], lhsT=ones_sb,
                                     rhs=acc[:, nf * FD:(nf + 1) * FD],
                                     start=True, stop=True)
                    nc.vector.reciprocal_approx_fast(
                        r_row[:, nf * FD:(nf + 1) * FD], sp[0:1, :])
                r_bc = rp.tile([P, N], F32, tag="rbc")
                nc.gpsimd.partition_broadcast(r_bc, r_row)

                out_group(0, 1)
                out_group(1, 0)
                out_group(1, 1)

                # -- normalize + bvo + residual + store --
                for (co, nf, po) in po_tiles:
                    dst = out_sb[:, co, nf * FD:(nf + 1) * FD]
                    nc.vector.tensor_tensor(dst, po,
                                            r_bc[:, nf * FD:(nf + 1) * FD],
                                            op=OP.mult)
                    nc.vector.scalar_tensor_tensor(
                        dst, dst, bvo_sb[:, co:co + 1],
                        x_sb[:, co, nf * FD:(nf + 1) * FD],
                        op0=OP.add, op1=OP.add)
                for co in range(CI):
                    nc.sync.dma_start(out_r[s][:, co, :], out_sb[:, co, :])

    nc.compile()
    return nc


_NC_CACHE = None


def _get_nc():
    global _NC_CACHE
    if _NC_CACHE is None:
        _NC_CACHE = build_nc()
    return _NC_CACHE


def _host_prep(wq, bq, wk, bk, wv, bv, wo, bo, gn_w, gn_b):
    f64 = np.float64
    wqT = np.ascontiguousarray(np.asarray(wq, f64).T.astype(ml_dtypes.bfloat16))
    wkT = np.ascontiguousarray(np.asarray(wk, f64).T.astype(ml_dtypes.bfloat16))
    wvo = np.asarray(wo, f64) @ np.asarray(wv, f64)
    wvoT = np.ascontiguousarray(wvo.T.astype(ml_dtypes.bfloat16))
    bvo = (np.asarray(wo, f64) @ np.asarray(bv, f64) + np.asarray(bo, f64)).astype(
        np.float32)

    # group-pooling selector: gsel[ci, c, g] = 1/8 if channel ci*P+c is in group g
    gsel = np.zeros((CI, P, G), np.float32)
    bsel = np.zeros((CI, G, P), np.float32)
    cpg = C // G
    for ci in range(CI):
        for c in range(P):
            g = (ci * P + c) // cpg
            gsel[ci, c, g] = 1.0 / cpg
            bsel[ci, g, c] = 1.0
    return dict(
        wqT=wqT, wkT=wkT, wvoT=wvoT,
        bq=np.asarray(bq, np.float32), bk=np.asarray(bk, np.float32), bvo=bvo,
        gnw=np.asarray(gn_w, np.float32), gnb=np.asarray(gn_b, np.float32),
        gsel=gsel, bsel=bsel, ones=np.ones((P, 1), np.float32),
    )


def kernel(x, gn_w, gn_b, wq, bq, wk, bk, wv, bv, wo, bo,
           _trace=False, _trace_kwargs=None):
    x = np.asarray(x, np.float32)
    assert x.shape == (B, C, 32, 32), x.shape
    shared = _host_prep(wq, bq, wk, bk, wv, bv, wo, bo, gn_w, gn_b)

    n_cores = B // B_LOC
    in_maps = []
    for core in range(n_cores):
        shard = np.ascontiguousarray(
            x[core * B_LOC:(core + 1) * B_LOC].reshape(B_LOC, C, N))
        in_maps.append({"x": shard, **shared})

    nc = _get_nc()
    res = run_bass_kernel_spmd(nc, in_maps, core_ids=list(range(n_cores)),
                               trace=_trace, **(_trace_kwargs or {}))
    out = np.concatenate(
        [res.results[i]["out"].reshape(B_LOC, C, 32, 32) for i in range(n_cores)],
        axis=0)
    kernel.last_results = res
    return out


# revision 11
# speedup vs baseline: 1.0234x; 1.0234x over previous
"""AttentionBlock (GroupNorm + single-head self-attention + residual) on Trainium2.

Reference computation (per sample, C=256 channels, N=H*W=1024 positions):
    h   = GroupNorm32(x) * gn_w + gn_b
    q   = wq @ h + bq;  k = wk @ h + bk;  v = wv @ h + bv      (1x1 convs)
    att = softmax((q^T k) * C^-0.5)                            [N, N]
    out = x + wo @ (att-weighted v) + bo

Sharding: data-parallel over batch B=32 across 8 NeuronCores (4 samples each).
Weights are replicated; everything per-sample stays on-chip.

Key algebraic rearrangements (exact, up to fp reassociation):
  * wo is folded into the v projection: vo = (wo@wv) @ h + (wo@bv + bo).
    Since softmax rows sum to 1, the constant (wo@bv + bo) passes through
    attention unchanged, so the final 1x1 conv disappears.
  * softmax is computed without max-subtraction (att*scale has |x| < ~1 here),
    and the division by the row sum is applied after the att @ vo matmul.
  * attention is computed transposed (attT[m, n]) so both bmms contract over
    the partition dimension; row sums are accumulated tile-wise on VectorE,
    then collapsed across partitions with a single M=1 ones-matmul per chunk.

Pipeline: all 4 samples' GroupNorm statistics are computed in a prologue so
the PE stream (projections -> attT -> att@vo) never waits on the
DVE->PE->ACT stats chain at sample boundaries (keeps the PE HAM-warm and
avoids per-sample Sqrt/Exp activation-table thrash).
"""

import sys

import ml_dtypes
import numpy as np

for _p in ("/opt/trn_rl_repo",):
    if _p not in sys.path:
        sys.path.insert(0, _p)

import concourse.bacc as bacc
import concourse.bass as bass
import concourse.tile as tile
from concourse import mybir
from concourse.bass_utils import run_bass_kernel_spmd

P = 128
B = 32
B_LOC = 4           # samples per core
C = 256
N = 1024            # H*W
CI = C // P         # 2 channel chunks (contraction side)
NT = N // P         # 8 spatial 128-tiles
FD = 512            # matmul free-dim chunk (one PSUM bank of fp32)
NF = N // FD        # 2 free chunks
G = 32              # groups
EPS = 1e-5
SCALE = float(C) ** -0.5
F32 = mybir.dt.float32
F32R = mybir.dt.float32r
BF16 = mybir.dt.bfloat16
AF = mybir.ActivationFunctionType
OP = mybir.AluOpType


def _r(ap):
    """View an fp32 AP as float32r for full-rate PE matmuls."""
    return ap.bitcast(F32R)


def build_nc():
    nc = bacc.Bacc("TRN2", debug=False, num_devices=8, enable_asserts=False)

    x_d = nc.dram_tensor("x", [B_LOC, C, N], F32, kind="ExternalInput").ap()
    wq_d = nc.dram_tensor("wqT", [C, C], BF16, kind="ExternalInput").ap()
    wk_d = nc.dram_tensor("wkT", [C, C], BF16, kind="ExternalInput").ap()
    wvo_d = nc.dram_tensor("wvoT", [C, C], BF16, kind="ExternalInput").ap()
    bq_d = nc.dram_tensor("bq", [C], F32, kind="ExternalInput").ap()
    bk_d = nc.dram_tensor("bk", [C], F32, kind="ExternalInput").ap()
    bvo_d = nc.dram_tensor("bvo", [C], F32, kind="ExternalInput").ap()
    gnw_d = nc.dram_tensor("gnw", [C], F32, kind="ExternalInput").ap()
    gnb_d = nc.dram_tensor("gnb", [C], F32, kind="ExternalInput").ap()
    gsel_d = nc.dram_tensor("gsel", [CI, P, G], F32, kind="ExternalInput").ap()
    ones_d = nc.dram_tensor("ones", [P, 1], F32R, kind="ExternalInput").ap()
    bsel_d = nc.dram_tensor("bsel", [CI, G, P], F32, kind="ExternalInput").ap()
    out_d = nc.dram_tensor("out", [B_LOC, C, N], F32, kind="ExternalOutput").ap()

    x_r = x_d.rearrange("b (ci p) n -> b p ci n", p=P)
    out_r = out_d.rearrange("b (co p) n -> b p co n", p=P)

    with tile.TileContext(nc) as tc:
        with (
            tc.tile_pool(name="const", bufs=1) as const,
            tc.tile_pool(name="xp", bufs=B_LOC) as xp,
            tc.tile_pool(name="hp", bufs=2) as hp,
            tc.tile_pool(name="qkp", bufs=2) as qkp,
            tc.tile_pool(name="vop", bufs=2) as vop,
            tc.tile_pool(name="attp", bufs=2) as attp,
            tc.tile_pool(name="outp", bufs=1) as outp,
            tc.tile_pool(name="accp", bufs=1) as accp,
            tc.tile_pool(name="smallp", bufs=2) as smallp,
            tc.tile_pool(name="stp", bufs=B_LOC) as stp,
            tc.tile_pool(name="rp", bufs=1) as rp,
            tc.tile_pool(name="psA", bufs=2, space="PSUM") as psA,  # proj/vo
            tc.tile_pool(name="psB", bufs=2, space="PSUM") as psB,  # att
            tc.tile_pool(name="psC", bufs=2, space="PSUM") as psC,  # out
        ):
            # ---------------- constants (loaded once) ----------------
            # Issued on the scalar-engine DGE queue (mostly idle at startup) in
            # dependency order: gsel/gn vectors feed the GroupNorm prologue,
            # weights are needed a few us later by the first projections.
            # x goes on the sync queue so the two streams don't serialize.
            gsel_sb = const.tile([P, CI, G], F32, tag="gsel")
            nc.scalar.dma_start(gsel_sb, gsel_d.rearrange("ci p g -> p ci g"))
            gnw_sb = const.tile([P, CI], F32, tag="gnw")
            nc.scalar.dma_start(gnw_sb, gnw_d.rearrange("(ci p) -> p ci", p=P))
            gnb_sb = const.tile([P, CI], F32, tag="gnb")
            nc.scalar.dma_start(gnb_sb, gnb_d.rearrange("(ci p) -> p ci", p=P))
            bsel_sb = const.tile([G, CI, P], F32, tag="bsel")
            nc.scalar.dma_start(bsel_sb, bsel_d.rearrange("ci g c -> g ci c"))
            wq_sb = const.tile([P, CI, C], BF16, tag="wq")
            nc.scalar.dma_start(wq_sb, wq_d.rearrange("(ci p) o -> p ci o", p=P))
            wk_sb = const.tile([P, CI, C], BF16, tag="wk")
            nc.scalar.dma_start(wk_sb, wk_d.rearrange("(ci p) o -> p ci o", p=P))
            wvo_sb = const.tile([P, CI, C], BF16, tag="wvo")
            nc.scalar.dma_start(wvo_sb, wvo_d.rearrange("(ci p) o -> p ci o", p=P))
            bq_sb = const.tile([P, CI], F32, tag="bq")
            nc.scalar.dma_start(bq_sb, bq_d.rearrange("(co p) -> p co", p=P))
            bk_sb = const.tile([P, CI], F32, tag="bk")
            nc.scalar.dma_start(bk_sb, bk_d.rearrange("(co p) -> p co", p=P))
            bvo_sb = const.tile([P, CI], F32, tag="bvo")
            nc.scalar.dma_start(bvo_sb, bvo_d.rearrange("(co p) -> p co", p=P))
            ones_sb = const.tile([P, 1], F32R, tag="ones")
            nc.scalar.dma_start(ones_sb, ones_d)
            eps_sb = const.tile([P, 1], F32, tag="eps")
            nc.vector.memset(eps_sb, EPS)

            # -------- prologue: load x + GroupNorm stats for ALL samples --------
            x_sbs = []
            st_sbs = []
            for s in range(B_LOC):
                x_sb = xp.tile([P, CI, N], F32, tag="x")
                nc.sync.dma_start(x_sb, x_r[s])
                x_sbs.append(x_sb)

            for s in range(B_LOC):
                x_sb = x_sbs[s]
                # per-channel (mean, var, mean^2) -> st3 [P, CI, 3]
                st3 = smallp.tile([P, CI, 3], F32, tag="st3")
                for ci in range(CI):
                    bnst = smallp.tile([P, 2, 6], F32, tag="bnst")
                    for sub in range(2):
                        nc.vector.bn_stats(
                            out=bnst[:, sub, :],
                            in_=x_sb[:, ci, sub * 512:(sub + 1) * 512],
                        )
                    nc.vector.bn_aggr(out=st3[:, ci, 0:2], in_=bnst)
                    nc.vector.tensor_mul(st3[:, ci, 2:3], st3[:, ci, 0:1],
                                         st3[:, ci, 0:1])
                # group-pooled: [G, 3] = (mean_g, E[var_c], E[m_c^2]) per group
                gstat_ps = psA.tile([G, 3], F32, tag="mm")
                for ci in range(CI):
                    nc.tensor.matmul(gstat_ps, lhsT=gsel_sb[:, ci, :],
                                     rhs=st3[:, ci, :],
                                     start=(ci == 0), stop=(ci == CI - 1))
                grp = smallp.tile([G, 2], F32, tag="grp")     # (mean_g, rstd_g)
                gtmp = smallp.tile([G, 2], F32, tag="gtmp")
                gst = smallp.tile([G, 3], F32, tag="gst")
                nc.vector.tensor_copy(gst, gstat_ps)
                nc.vector.tensor_add(gtmp[:, 0:1], gst[:, 1:2], gst[:, 2:3])
                nc.vector.tensor_mul(gtmp[:, 1:2], gst[:, 0:1], gst[:, 0:1])
                nc.vector.tensor_sub(gtmp[:, 0:1], gtmp[:, 0:1], gtmp[:, 1:2])
                nc.vector.tensor_copy(grp[:, 0:1], gst[:, 0:1])
                nc.scalar.activation(out=gtmp[:, 1:2], in_=gtmp[:, 0:1],
                                     func=AF.Sqrt, bias=eps_sb[:G], scale=1.0)
                nc.vector.reciprocal(grp[:, 1:2], gtmp[:, 1:2])

                # broadcast group -> channels; per-channel scale/shift (s_c, t_c)
                st = stp.tile([P, CI, 2], F32, tag="st")
                for ci in range(CI):
                    chan_ps = psA.tile([P, 2], F32, tag="mm")
                    nc.tensor.matmul(chan_ps, lhsT=bsel_sb[:, ci, :], rhs=grp,
                                     start=True, stop=True)
                    nc.vector.tensor_mul(st[:, ci, 0:1], chan_ps[:, 1:2],
                                         gnw_sb[:, ci:ci + 1])
                    nc.vector.tensor_mul(st[:, ci, 1:2], chan_ps[:, 0:1],
                                         st[:, ci, 0:1])
                    nc.vector.tensor_sub(st[:, ci, 1:2], gnb_sb[:, ci:ci + 1],
                                         st[:, ci, 1:2])
                st_sbs.append(st)

            def compute_h(s):
                """h = x * s_c + t_c on ScalarE, written as f32r."""
                h_sb = hp.tile([P, CI, N], BF16, tag="h")
                for ci in range(CI):
                    nc.scalar.activation(out=h_sb[:, ci, :], in_=x_sbs[s][:, ci, :],
                                         func=AF.Identity,
                                         bias=st_sbs[s][:, ci, 1:2],
                                         scale=st_sbs[s][:, ci, 0:1])
                return h_sb

            h_next = compute_h(0)

            # ---------------- per-sample main pipeline ----------------
            for s in range(B_LOC):
                x_sb = x_sbs[s]
                h_sb = h_next

                # -- projections: qT/k in [C, N] layout, vo in [N, C] layout --
                qT_sb = qkp.tile([P, CI, N], BF16, tag="qT")
                k_sb = qkp.tile([P, CI, N], BF16, tag="k")
                for (w_sb, b_sb, dst) in ((wq_sb, bq_sb, qT_sb),
                                          (wk_sb, bk_sb, k_sb)):
                    for co in range(CI):
                        for nf in range(NF):
                            ps = psA.tile([P, FD], F32, tag="mm")
                            for ci in range(CI):
                                nc.tensor.matmul(
                                    ps,
                                    lhsT=w_sb[:, ci, co * P:(co + 1) * P],
                                    rhs=h_sb[:, ci, nf * FD:(nf + 1) * FD],
                                    start=(ci == 0), stop=(ci == CI - 1))
                            nc.scalar.activation(
                                out=dst[:, co, nf * FD:(nf + 1) * FD], in_=ps,
                                func=AF.Identity, bias=b_sb[:, co:co + 1],
                                scale=1.0)

                vo_sb = vop.tile([P, NT, C], BF16, tag="vo")
                for nt in range(NT):
                    ps = psA.tile([P, FD], F32, tag="mm")
                    for ci in range(CI):
                        nc.tensor.matmul(
                            ps[:, 0:C],
                            lhsT=h_sb[:, ci, nt * P:(nt + 1) * P],
                            rhs=wvo_sb[:, ci, :],
                            start=(ci == 0), stop=(ci == CI - 1))
                    nc.vector.tensor_copy(vo_sb[:, nt, :], ps[:, 0:C])

                # -- attT = k^T q, exp on the fly (ScalarE) --
                ax_sb = attp.tile([P, NT, N], BF16, tag="ax")
                acc = accp.tile([P, N], F32R, tag="acc")
                for mt in range(NT):
                    ps = psB.tile([P, N], F32, tag="att")
                    for nf in range(NF):
                        for ci in range(CI):
                            nc.tensor.matmul(
                                ps[:, nf * FD:(nf + 1) * FD],
                                lhsT=k_sb[:, ci, mt * P:(mt + 1) * P],
                                rhs=qT_sb[:, ci, nf * FD:(nf + 1) * FD],
                                start=(ci == 0), stop=(ci == CI - 1))
                    nc.scalar.activation(
                        out=ax_sb[:, mt, :], in_=ps,
                        func=AF.Exp, bias=0.0, scale=SCALE)
                    # running softmax row-sum accumulation (VectorE), kept
                    # tight behind the exps so the reciprocal/broadcast chain
                    # finishes before the att@vo PSUM slots are needed.
                    if mt == 1:
                        nc.vector.tensor_add(acc, ax_sb[:, 0, :], ax_sb[:, 1, :])
                    elif mt > 1:
                        nc.vector.tensor_add(acc, acc, ax_sb[:, mt, :])

                # h for the NEXT sample: emitted here so ScalarE computes it
                # while the PE streams this sample's att/out matmuls.
                if s + 1 < B_LOC:
                    h_next = compute_h(s + 1)

                # -- out_pre[c, n] = sum_m vo[m, c] * ax[m, n] --
                # First out-matmul group streams while the acc adds finish;
                # the sums collapse + reciprocal + broadcast are emitted right
                # after it so the merge inputs are ready before the remaining
                # groups retire.
                out_sb = outp.tile([P, CI, N], F32, tag="out")
                po_tiles = []

                def out_group(co, nf):
                    po = psC.tile([P, FD], F32, tag="o")
                    for mt in range(NT):
                        nc.tensor.matmul(
                            po,
                            lhsT=vo_sb[:, mt, co * P:(co + 1) * P],
                            rhs=ax_sb[:, mt, nf * FD:(nf + 1) * FD],
                            start=(mt == 0), stop=(mt == NT - 1))
                    po_tiles.append((co, nf, po))

                out_group(0, 0)

                # -- collapse row sums across partitions; reciprocal; broadcast --
                r_row = rp.tile([1, N], F32, tag="rrow")
                for nf in range(NF):
                    sp = psA.tile([P, FD], F32, tag="mm")
                    nc.tensor.matmul(sp[0:1, :], lhsT=ones_sb,
                                     rhs=acc[:, nf * FD:(nf + 1) * FD],
                                     start=True, stop=True)
                    nc.vector.reciprocal_approx_fast(
                        r_row[:, nf * FD:(nf + 1) * FD], sp[0:1, :])
                r_bc = rp.tile([P, N], F32, tag="rbc")
                nc.gpsimd.partition_broadcast(r_bc, r_row)

                out_group(0, 1)
                out_group(1, 0)
                out_group(1, 1)

                # -- normalize + bvo + residual + store --
                for (co, nf, po) in po_tiles:
                    dst = out_sb[:, co, nf * FD:(nf + 1) * FD]
                    nc.vector.tensor_tensor(dst, po,
                                            r_bc[:, nf * FD:(nf + 1) * FD],
                                            op=OP.mult)
                    nc.vector.scalar_tensor_tensor(
                        dst, dst, bvo_sb[:, co:co + 1],
                        x_sb[:, co, nf * FD:(nf + 1) * FD],
                        op0=OP.add, op1=OP.add)
                for co in range(CI):
                    nc.sync.dma_start(out_r[s][:, co, :], out_sb[:, co, :])

    nc.compile()
    return nc


_NC_CACHE = None


def _get_nc():
    global _NC_CACHE
    if _NC_CACHE is None:
        _NC_CACHE = build_nc()
    return _NC_CACHE


def _host_prep(wq, bq, wk, bk, wv, bv, wo, bo, gn_w, gn_b):
    f64 = np.float64
    wqT = np.ascontiguousarray(np.asarray(wq, f64).T.astype(ml_dtypes.bfloat16))
    wkT = np.ascontiguousarray(np.asarray(wk, f64).T.astype(ml_dtypes.bfloat16))
    wvo = np.asarray(wo, f64) @ np.asarray(wv, f64)
    wvoT = np.ascontiguousarray(wvo.T.astype(ml_dtypes.bfloat16))
    bvo = (np.asarray(wo, f64) @ np.asarray(bv, f64) + np.asarray(bo, f64)).astype(
        np.float32)

    # group-pooling selector: gsel[ci, c, g] = 1/8 if channel ci*P+c is in group g
    gsel = np.zeros((CI, P, G), np.float32)
    bsel = np.zeros((CI, G, P), np.float32)
    cpg = C // G
    for ci in range(CI):
        for c in range(P):
            g = (ci * P + c) // cpg
            gsel[ci, c, g] = 1.0 / cpg
            bsel[ci, g, c] = 1.0
    return dict(
        wqT=wqT, wkT=wkT, wvoT=wvoT,
        bq=np.asarray(bq, np.float32), bk=np.asarray(bk, np.float32), bvo=bvo,
        gnw=np.asarray(gn_w, np.float32), gnb=np.asarray(gn_b, np.float32),
        gsel=gsel, bsel=bsel, ones=np.ones((P, 1), np.float32),
    )


def kernel(x, gn_w, gn_b, wq, bq, wk, bk, wv, bv, wo, bo,
           _trace=False, _trace_kwargs=None):
    x = np.asarray(x, np.float32)
    assert x.shape == (B, C, 32, 32), x.shape
    shared = _host_prep(wq, bq, wk, bk, wv, bv, wo, bo, gn_w, gn_b)

    n_cores = B // B_LOC
    in_maps = []
    for core in range(n_cores):
        shard = np.ascontiguousarray(
            x[core * B_LOC:(core + 1) * B_LOC].reshape(B_LOC, C, N))
        in_maps.append({"x": shard, **shared})

    nc = _get_nc()
    res = run_bass_kernel_spmd(nc, in_maps, core_ids=list(range(n_cores)),
                               trace=_trace, **(_trace_kwargs or {}))
    out = np.concatenate(
        [res.results[i]["out"].reshape(B_LOC, C, 32, 32) for i in range(n_cores)],
        axis=0)
    kernel.last_results = res
    return out
